# revision 16
# baseline (speedup 1.0000x reference)
"""Trainium2 Bass kernel for batched multi-head self-attention block.

Full-input contract: kernel(**inputs) takes the complete tensors
(x [2,2048,1024], Wqkv [1024,3072], bqkv [3072], Wout [1024,1024], bout [1024])
and returns the full output [2,2048,1024].

Sharding: 8 cores = 2 (batch, data parallel) x 4 (head groups of 4 heads,
tensor parallel over the qkv/out projections). Each core computes a partial
output [2048,1024] for its batch; host sums the 4 head-group partials per
batch and adds bout.

All matmuls single-pass fp16 (rel-err budget 2e-2 allows ~1e-3 fp16 error).
Attention scores row-pack head pairs (HD=64 contraction -> PE row halves),
softmax exp runs as one [128,1024] ScalarE activation per head-pair slot.
The timing loop body is unrolled 2x with ping-pong qk/v buffers so the
next step's projections overlap the current step's (ScalarE-bound)
attention phase.
"""

import numpy as np

B, T, D, H, HD = 2, 2048, 1024, 16, 64
NCORES = 8
NHEADS = 4            # heads per core
NQK = NHEADS * HD     # 256
TQB = 512             # q block size
NBLK = T // TQB       # 4
DT = D // 128         # 8 d-tiles
TT = T // 128         # 16 t-tiles
TKT = T // 128        # 16 tk-tiles


def _patch_tile_drain():
    """walrus CoreV3 rejects >2 sem waits on one CTRL instruction; split the
    Tile kernel-tail drain waits across single-wait nops."""
    import concourse.tile as tile
    import concourse.mybir as mybir
    from concourse.vector_clock import ScopedClock

    if getattr(tile.TileContext, "_drain_patched", False):
        return

    def _drain_and_barrier_split(self, tick_clock, wait_clock):
        nc = self.nc
        drain_inst = nc.sync.drain()
        wait_clock.add_sem_waits(
            drain_inst.ins, ScopedClock({None: tick_clock.global_clock})
        )
        mi = drain_inst.ins
        si = getattr(mi, "sync_info", None)
        waits = list(si.on_wait or []) if si is not None else []
        if len(waits) > 1:
            si.on_wait = waits[:1]
            for w in waits[1:]:
                nop = nc.sync.nop().ins
                if getattr(nop, "sync_info", None) is None:
                    nop.sync_info = mybir.SyncInfo(on_wait=[w], on_update=[])
                else:
                    nop.sync_info.on_wait = [w]

        nc.all_engine_barrier()
        assert self.sems is not None
        popped = nc._tile_sem_poison_stack.pop()
        assert popped is self._sem_poison
        nc.clear_and_free_semaphores(list(self.sems.allocated().values()))
        nc.all_engine_barrier()

    tile.TileContext._drain_and_barrier = _drain_and_barrier_split
    tile.TileContext._drain_patched = True


def split_excess_waits(nc, max_waits=1):
    """walrus CoreV3 in this env accepts at most 1 sync-wait per instruction;
    move extras onto same-engine nops inserted just before."""
    import concourse.mybir as mybir

    ctr = 0
    for f in nc.m.functions:
        for b in f.blocks:
            newlist = []
            changed = False
            for inst in b.instructions:
                si = getattr(inst, "sync_info", None)
                waits = list(si.on_wait or []) if si is not None else []
                if len(waits) > max_waits:
                    assert inst.engine != mybir.EngineType.Unassigned, inst
                    for w in waits[:-max_waits]:
                        ctr += 1
                        nop = mybir.InstNoOp(name=f"waitnop-{ctr}", ins=[], outs=[])
                        nop.engine = inst.engine
                        nop.sync_info = mybir.SyncInfo(on_wait=[w], on_update=[])
                        newlist.append(nop)
                    si.on_wait = waits[-max_waits:]
                    changed = True
                newlist.append(inst)
            if changed:
                b.instructions = newlist
    return ctr


def build_nc(loop_n=None):
    import concourse.bass as bass
    import concourse.mybir as mybir
    import concourse.tile as tile
    from contextlib import ExitStack

    _patch_tile_drain()
    f32 = mybir.dt.float32
    f16 = mybir.dt.float16
    EXP = mybir.ActivationFunctionType.Exp

    from concourse.tile_rust import add_dep_helper

    def chain(mms):
        for a, b_ in zip(mms[1:], mms[:-1]):
            add_dep_helper(a.ins, b_.ins, sync=False, reason="psum group order")

    nc = bass.Bass()
    x16d = nc.declare_dram_parameter("x16", [T, D], f16, isOutput=False)
    wqkd = nc.declare_dram_parameter("wqk16", [D, 2 * NQK], f16, isOutput=False)
    wvd = nc.declare_dram_parameter("wv16", [D, NQK], f16, isOutput=False)
    woutd = nc.declare_dram_parameter("wout16", [NQK, D], f16, isOutput=False)
    bqkd = nc.declare_dram_parameter("bqk", [2 * NQK], f32, isOutput=False)
    bvd = nc.declare_dram_parameter("bv", [1, NQK], f32, isOutput=False)
    outd = nc.declare_dram_parameter("out", [T, D], f16, isOutput=True)

    screc = nc.dram_tensor("screc", [4 * NBLK, TQB], f32)

    with tile.TileContext(nc) as tc, ExitStack() as ctx:
        const_p = ctx.enter_context(tc.tile_pool(name="const", bufs=1))
        big_p = ctx.enter_context(tc.tile_pool(name="big", bufs=1))

        wqk16 = const_p.tile([128, DT, 2 * NQK], f16, tag="wqk16")
        wv16 = const_p.tile([128, DT, NQK], f16, tag="wv16")
        wout16 = const_p.tile([128, 2, D], f16, tag="wout16")
        nc.sync.dma_start(out=wqk16, in_=wqkd.rearrange("(dt p) n -> p dt n", p=128))
        nc.sync.dma_start(out=wv16, in_=wvd.rearrange("(dt p) n -> p dt n", p=128))
        nc.sync.dma_start(out=wout16, in_=woutd.rearrange("(kt p) n -> p kt n", p=128))
        bqk_sb = const_p.tile([128, 4], f32, tag="bqk")
        nc.sync.dma_start(out=bqk_sb, in_=bqkd.rearrange("(m p) -> p m", p=128))
        # v bias broadcast across partitions (for fused add in the v drain)
        bvb = const_p.tile([128, NQK], f32, tag="bvb")
        import concourse.bass as _b
        nc.sync.dma_start(
            out=bvb,
            in_=_b.AP(tensor=bvd[:, :].tensor, offset=0, ap=[[0, 128], [1, NQK]]),
        )

        # persistent activations (qk/vaug ping-pong for the unrolled loop)
        nab = 2 if loop_n else 1
        qk16s = [
            big_p.tile([128, 4, T], f16, tag=f"qk16_{i}", name=f"qk16_{i}")
            for i in range(nab)
        ]
        vaug16s = [
            big_p.tile(
                [128, TT, 4 * (HD + 1)], f16, tag=f"vaug16_{i}",
                name=f"vaug16_{i}",
            )
            for i in range(nab)
        ]
        xt16s = [
            big_p.tile([128, DT, T], f16, tag=f"xt16_{i}", name=f"xt16_{i}")
            for i in range(nab)
        ]
        cxt16s = [
            big_p.tile([128, 2, T], f16, tag=f"cxt16_{i}", name=f"cxt16_{i}")
            for i in range(nab)
        ]
        rb = big_p.tile([128, T], f32, tag="rb")               # recip bcast
        scol = big_p.tile([4 * NBLK, TQB], f32, tag="scol")    # softmax denom
        rec = big_p.tile([4 * NBLK, TQB], f32, tag="rec")

        # ones columns of v_aug (once; v writes never touch col 64)
        for vaug16 in vaug16s:
            nc.vector.memset(
                vaug16.rearrange("p t (h c) -> p t h c", h=4)[:, :, :, HD : HD + 1],
                1.0,
            )

        # persistent PSUM pools: 1 + 1 + 4 + 2 = 8 banks
        qk_ps_p = ctx.enter_context(
            tc.tile_pool(name="qkps", bufs=1, space="PSUM")
        )
        v_ps_p = ctx.enter_context(tc.tile_pool(name="vps", bufs=1, space="PSUM"))
        sp_ps_p = ctx.enter_context(tc.tile_pool(name="sps", bufs=2, space="PSUM"))
        cp_ps_p = ctx.enter_context(tc.tile_pool(name="cps", bufs=2, space="PSUM"))

        at_p = ctx.enter_context(tc.tile_pool(name="atp", bufs=16))
        ot_p = ctx.enter_context(tc.tile_pool(name="otp", bufs=4))

        def ph0(xt):
            for dt in range(DT):
                nc.sync.dma_start_transpose(
                    xt[:, dt, :], x16d[:, dt * 128 : (dt + 1) * 128]
                )

        def qk_group(xt, qk16, m, cb):
            ps = qk_ps_p.tile([128, TQB], f32, tag="qkps", name="ps")
            mms = []
            for dt in range(DT):
                mms.append(nc.tensor.matmul(
                    ps,
                    lhsT=wqk16[:, dt, m * 128 : (m + 1) * 128],
                    rhs=xt[:, dt, cb * TQB : (cb + 1) * TQB],
                    start=(dt == 0),
                    stop=(dt == DT - 1),
                    skip_group_check=True,
                ))
            chain(mms)
            nc.vector.tensor_scalar_add(
                out=qk16[:, m, cb * TQB : (cb + 1) * TQB],
                in0=ps,
                scalar1=bqk_sb[:, m : m + 1],
            )

        def v_group(xt, vaug16, tt):
            ps = v_ps_p.tile([128, NQK], f32, tag="vps", name="ps")
            mms = []
            for dt in range(DT):
                mms.append(nc.tensor.matmul(
                    ps,
                    lhsT=xt[:, dt, tt * 128 : (tt + 1) * 128],
                    rhs=wv16[:, dt, :],
                    start=(dt == 0),
                    stop=(dt == DT - 1),
                    skip_group_check=True,
                ))
            chain(mms)
            nc.vector.tensor_add(
                out=vaug16.rearrange("p t (h c) -> p t h c", h=4)[
                    :, tt, :, 0:HD
                ],
                in0=ps.rearrange("p (h c) -> p h c", h=4),
                in1=bvb.rearrange("p (h c) -> p h c", h=4),
            )

        def ph1_chunks(xt, qk16, vaug16):
            chunks = []
            for m in (2, 0):
                for cb in range(4):
                    chunks.append(
                        lambda m=m, cb=cb: qk_group(xt, qk16, m, cb)
                    )
            for tt in range(TT):
                chunks.append(lambda tt=tt: v_group(xt, vaug16, tt))
            for m in (3, 1):
                for cb in range(4):
                    chunks.append(
                        lambda m=m, cb=cb: qk_group(xt, qk16, m, cb)
                    )
            return chunks

        def ph1(xt, qk16, vaug16):
            for c in ph1_chunks(xt, qk16, vaug16):
                c()

        def out_group(cxt16, tt, nb):
            ops = cp_ps_p.tile([128, TQB], f32, tag="cps", name="ops")
            mms = []
            for kt in range(2):
                mms.append(nc.tensor.matmul(
                    ops,
                    lhsT=cxt16[:, kt, tt * 128 : (tt + 1) * 128],
                    rhs=wout16[:, kt, nb * TQB : (nb + 1) * TQB],
                    start=(kt == 0),
                    stop=(kt == 1),
                    skip_group_check=True,
                ))
            chain(mms)
            ot = ot_p.tile([128, TQB], f16, tag="ot", name="ot")
            nc.vector.tensor_copy(ot, ops)
            nc.sync.dma_start(
                out=outd[
                    tt * 128 : (tt + 1) * 128,
                    nb * TQB : (nb + 1) * TQB,
                ],
                in_=ot,
            )

        def ph3_chunks(cxt16):
            return [
                lambda tt=tt, nb=nb: out_group(cxt16, tt, nb)
                for tt in range(TT)
                for nb in range(2)
            ]

        def ph3(cxt16):
            for c in ph3_chunks(cxt16):
                c()

        def ph2(qk16, vaug16, cxt16, slot_fillers=(), blk_fillers=()):
            sf = list(slot_fillers)
            bf = list(blk_fillers)
            for hp in range(2):
                h0, h1 = 2 * hp, 2 * hp + 1
                qtile, ktile = hp, 2 + hp
                for blk in range(NBLK):
                    cps0 = cp_ps_p.tile([HD + 1, TQB], f32, tag="cps", name="cps0")
                    cps1 = cp_ps_p.tile([HD + 1, TQB], f32, tag="cps", name="cps1")
                    cm0s, cm1s = [], []
                    prev = None
                    for tk in range(TKT):
                        sps = sp_ps_p.tile(
                            [128, 2, TQB], f32, tag="sps", name="sps"
                        )
                        nc.tensor.matmul(
                            sps[:, 0, :],
                            lhsT=qk16[0:64, ktile, tk * 128 : (tk + 1) * 128],
                            rhs=qk16[0:64, qtile, blk * TQB : (blk + 1) * TQB],
                            start=True,
                            stop=True,
                            skip_group_check=True,
                            tile_position=(0, 0),
                        )
                        nc.tensor.matmul(
                            sps[:, 1, :],
                            lhsT=qk16[64:128, ktile, tk * 128 : (tk + 1) * 128],
                            rhs=qk16[64:128, qtile, blk * TQB : (blk + 1) * TQB],
                            start=True,
                            stop=True,
                            skip_group_check=True,
                            tile_position=(64, 0),
                        )
                        at = at_p.tile([128, 2, TQB], f16, tag="at", name="at")
                        nc.scalar.activation(at, sps, EXP, scale=0.125)
                        # ctx matmuls run one slot behind their exp so they
                        # never head-of-line block the next score pair in the
                        # PE queue
                        if prev is not None:
                            pat, ptk = prev
                            cm0s.append(nc.tensor.matmul(
                                cps0,
                                lhsT=vaug16[
                                    :, ptk, h0 * (HD + 1) : (h0 + 1) * (HD + 1)
                                ],
                                rhs=pat[:, 0, :],
                                start=(ptk == 0),
                                stop=False,
                                skip_group_check=True,
                            ))
                            cm1s.append(nc.tensor.matmul(
                                cps1,
                                lhsT=vaug16[
                                    :, ptk, h1 * (HD + 1) : (h1 + 1) * (HD + 1)
                                ],
                                rhs=pat[:, 1, :],
                                start=(ptk == 0),
                                stop=False,
                                skip_group_check=True,
                            ))
                        prev = (at, tk)
                        if tk % 2 == 1 and not (hp == 0 and blk == 0) and sf:
                            sf.pop(0)()
                    pat, ptk = prev
                    cm0s.append(nc.tensor.matmul(
                        cps0,
                        lhsT=vaug16[:, ptk, h0 * (HD + 1) : (h0 + 1) * (HD + 1)],
                        rhs=pat[:, 0, :],
                        start=False,
                        stop=True,
                        skip_group_check=True,
                    ))
                    cm1s.append(nc.tensor.matmul(
                        cps1,
                        lhsT=vaug16[:, ptk, h1 * (HD + 1) : (h1 + 1) * (HD + 1)],
                        rhs=pat[:, 1, :],
                        start=False,
                        stop=True,
                        skip_group_check=True,
                    ))
                    chain(cm0s)
                    chain(cm1s)
                    # drain ctx + softmax denominators
                    nc.vector.tensor_copy(
                        out=cxt16[0:64, hp, blk * TQB : (blk + 1) * TQB],
                        in_=cps0[0:HD, :],
                    )
                    nc.vector.tensor_copy(
                        out=cxt16[64:128, hp, blk * TQB : (blk + 1) * TQB],
                        in_=cps1[0:HD, :],
                    )
                    r0 = h0 * NBLK + blk
                    r1 = h1 * NBLK + blk
                    stg0 = at_p.tile([1, TQB], f32, tag="stg", bufs=4, name="stg0")
                    nc.vector.tensor_copy(out=stg0, in_=cps0[HD : HD + 1, :])
                    nc.sync.dma_start(out=scol[r0 : r0 + 1, :], in_=stg0)
                    stg1 = at_p.tile([1, TQB], f32, tag="stg", bufs=4, name="stg1")
                    nc.vector.tensor_copy(out=stg1, in_=cps1[HD : HD + 1, :])
                    nc.sync.dma_start(out=scol[r1 : r1 + 1, :], in_=stg1)
                    # previous step's out-projection groups at blk boundaries
                    for _ in range(4):
                        if bf:
                            bf.pop(0)()
            for c in sf:
                c()
            for c in bf:
                c()

        def norm(cxt16):
            nc.vector.reciprocal(rec, scol)
            nc.sync.dma_start(out=screc[:, :], in_=rec)
            for kt in range(2):
                bsrc = _b.AP(
                    tensor=screc[:].tensor,
                    offset=kt * 2 * T,
                    ap=[[T, 2], [0, 64], [1, T]],
                )
                nc.sync.dma_start(out=rb, in_=bsrc)
                nc.vector.tensor_mul(cxt16[:, kt, :], cxt16[:, kt, :], rb)

        if loop_n:
            assert loop_n % 2 == 0, "loop_n must be even (2x unrolled body)"
            # prologue: first A-side inputs + init cxt16 B so iteration 0's
            # B-output fills read initialized data
            ph0(xt16s[0])
            ph1(xt16s[0], qk16s[0], vaug16s[0])
            nc.vector.memset(cxt16s[1], 0.01)
            nc.vector.memset(scol, 1.0)
            with tc.For_i(0, loop_n // 2, 1):
                ph0(xt16s[1])
                ph2(
                    qk16s[0], vaug16s[0], cxt16s[0],
                    slot_fillers=ph1_chunks(xt16s[1], qk16s[1], vaug16s[1]),
                    blk_fillers=ph3_chunks(cxt16s[1]),
                )
                norm(cxt16s[0])
                ph0(xt16s[0])
                ph2(
                    qk16s[1], vaug16s[1], cxt16s[1],
                    slot_fillers=ph1_chunks(xt16s[0], qk16s[0], vaug16s[0]),
                    blk_fillers=ph3_chunks(cxt16s[0]),
                )
                norm(cxt16s[1])
            # epilogue: final B-side output
            ph3(cxt16s[1])
        else:
            ph0(xt16s[0])
            ph1(xt16s[0], qk16s[0], vaug16s[0])
            ph2(qk16s[0], vaug16s[0], cxt16s[0])
            norm(cxt16s[0])
            ph3(cxt16s[0])

    return nc


_NC_CACHE = None


def _get_nc():
    global _NC_CACHE
    if _NC_CACHE is None:
        nc = build_nc()
        split_excess_waits(nc)
        _NC_CACHE = nc
    return _NC_CACHE


def make_in_maps(x, Wqkv, bqkv, Wout):
    x = np.asarray(x, dtype=np.float32)
    Wqkv = np.asarray(Wqkv, dtype=np.float32)
    bqkv = np.asarray(bqkv, dtype=np.float32)
    Wout = np.asarray(Wout, dtype=np.float32)
    in_maps = []
    for c in range(NCORES):
        b, g = divmod(c, 4)
        qs = slice(NQK * g, NQK * (g + 1))
        ks = slice(D + NQK * g, D + NQK * (g + 1))
        vs = slice(2 * D + NQK * g, 2 * D + NQK * (g + 1))
        in_maps.append(
            {
                "x16": np.ascontiguousarray(x[b].astype(np.float16)),
                "wqk16": np.ascontiguousarray(
                    np.concatenate([Wqkv[:, qs], Wqkv[:, ks]], axis=1).astype(
                        np.float16
                    )
                ),
                "wv16": np.ascontiguousarray(Wqkv[:, vs].astype(np.float16)),
                "wout16": np.ascontiguousarray(
                    Wout[NQK * g : NQK * (g + 1), :].astype(np.float16)
                ),
                "bqk": np.ascontiguousarray(
                    np.concatenate([bqkv[qs], bqkv[ks]])
                ),
                "bv": np.ascontiguousarray(bqkv[vs]).reshape(1, NQK).astype(
                    np.float32
                ),
            }
        )
    return in_maps


def gather_out(results, bout):
    bout = np.asarray(bout, dtype=np.float32)
    outs = [
        np.asarray(results[c]["out"], dtype=np.float32) for c in range(NCORES)
    ]
    full = np.stack(
        [outs[4 * b] + outs[4 * b + 1] + outs[4 * b + 2] + outs[4 * b + 3]
         for b in range(B)]
    )
    return (full + bout[None, None, :]).astype(np.float32)


def kernel(x, Wqkv, bqkv, Wout, bout):
    from concourse.bass_utils import run_bass_kernel_spmd

    nc = _get_nc()
    in_maps = make_in_maps(x, Wqkv, bqkv, Wout)
    res = run_bass_kernel_spmd(nc, in_maps, list(range(NCORES)))
    return gather_out(res.results, bout)
